# revision 6
# baseline (speedup 1.0000x reference)
"""TRN2 Bass kernel for nn_Attention_1709396984084.

Reference computation (per batch element b, 8 of them -> one NeuronCore each):
    x  = x_b @ lin_w.T + lin_b          # [S, D]
    Q  = x @ W_q ; K = x @ W_k ; V = x @ W_v
    I  = Q @ K.T  (causal masked, NO 1/sqrt(d) scaling)
    F  = softmax(I) @ V
    returns (F, stack([K, V]))

Layout strategy (hardcoded for S=2048, D=H=1024, B=8, data-parallel over batch):
  - host passes xT = x_b.T [D, S] and lwT = lin_w.T so stage 1 needs no
    on-device transposes; stage 1 produces xpT [e, s] (e on partitions),
    which feeds the Q/K projections as moving operand (-> Q^T, K^T layouts)
    and the V projection as stationary operand (-> V natural layout).
  - Q^T spills to DRAM and streams back per 128-query chunk (SBUF capacity);
    K^T [h, s] f32 and V bf16 [s, h] stay resident.
  - scores = Q^T.T @ K^T land [q, k] in PSUM; row softmax = free-dim
    reduce_max + ScalarE exp with per-partition -max bias and fused row-sum
    (accum_out); P~ cast to bf16, transposed 128x128 on TensorE (identity),
    then P~^T @ V accumulates F; 1/rowsum applied on the way out.
  - matmul dtypes: float32r (fp32 storage, ~1.5e-4 matmul rel err, bf16-rate
    at N=512) for the logit-sensitive chain; bf16 for P@V.
  - attention runs q-chunks in DESCENDING order (big chunks first) with a
    one-chunk software pipeline so P@V of chunk i fills the PE while the
    softmax of chunk i-1 runs on DVE/ACT.
Measured: relF ~2.8e-3, relK/V ~2e-4 vs the fp32 reference.
"""

import sys

sys.path.insert(0, "/opt/trn_rl_repo")

import numpy as np

P = 128
S = 2048  # sequence length
D = 1024  # input size
H = 1024  # hidden size
FT = 512  # free-dim tile (one PSUM bank of fp32)
NB = 8  # batch == number of cores
DC = D // P  # 8 contraction chunks
HC = H // P
ST = S // FT  # 4 s-tiles
QC = S // P  # 16 query chunks
NEG = -1.0e30

_cache = {}


def _build():
    import concourse.mybir as mybir
    import concourse.tile as tile
    from concourse import bacc
    from concourse.masks import make_identity

    f32 = mybir.dt.float32
    f32r = mybir.dt.float32r
    bf16 = mybir.dt.bfloat16
    EXP = mybir.ActivationFunctionType.Exp
    AX = mybir.AxisListType.X

    def r(ap):
        return ap

    nc = bacc.Bacc(None, target_bir_lowering=False)

    xT_d = nc.declare_dram_parameter("xT", [D, S], f32r, isOutput=False)
    lwT_d = nc.declare_dram_parameter("lwT", [D, D], f32r, isOutput=False)
    lb_d = nc.declare_dram_parameter("lin_b", [P, DC], f32, isOutput=False)
    wq_d = nc.declare_dram_parameter("W_q", [D, H], f32r, isOutput=False)
    wk_d = nc.declare_dram_parameter("W_k", [D, H], f32r, isOutput=False)
    wv_d = nc.declare_dram_parameter("W_v", [D, H], f32r, isOutput=False)
    mask_d = nc.declare_dram_parameter("masks", [4, P, FT], f32, isOutput=False)
    F_d = nc.declare_dram_parameter("F_out", [S, H], f32, isOutput=True)
    KT_d = nc.declare_dram_parameter("KT_out", [H, S], f32r, isOutput=True)
    V_d = nc.declare_dram_parameter("V_out", [S, H], f32r, isOutput=True)
    qt_spill = nc.dram_tensor("QT_spill", [H, S], f32r)

    with tile.TileContext(nc) as tc:
        xptp = tc.alloc_tile_pool(name="xptp", bufs=1)
        stg = tc.alloc_tile_pool(name="stg", bufs=3)
        wchp = tc.alloc_tile_pool(name="wchp", bufs=8)
        psmm = tc.alloc_tile_pool(name="psmm", bufs=8, space="PSUM")
        xp_sb = xptp.tile([P, DC, S], f32r, tag="xpt", name="xpt")

        # ---- stage 1: xpT[e, s] = lin_w @ x.T + b ----
        xin = tc.alloc_tile_pool(name="xin", bufs=1)
        xt_sb = xin.tile([P, DC, S], f32r, tag="xt", name="xt")
        lw_sb = xin.tile([P, DC, D], f32r, tag="lwt", name="lwt")
        bias_sb = xin.tile([P, DC], f32, tag="bias", name="bias")
        nc.sync.dma_start(bias_sb[:], lb_d.ap())
        for dc in range(DC):
            for st in range(ST):  # split across queues for early arrival
                nc.sync.dma_start(
                    xt_sb[:, dc, st * FT : (st + 1) * FT],
                    xT_d.ap()[dc * P : (dc + 1) * P, st * FT : (st + 1) * FT],
                )
            nc.sync.dma_start(
                lw_sb[:, dc, 0:FT], lwT_d.ap()[dc * P : (dc + 1) * P, 0:FT]
            )
            nc.sync.dma_start(
                lw_sb[:, dc, FT:D], lwT_d.ap()[dc * P : (dc + 1) * P, FT:D]
            )
        for ec in range(DC):
            pts = [psmm.tile([P, FT], f32, tag="mm", name="mm") for _ in range(ST)]
            for dc in range(DC):
                for st in range(ST):
                    nc.tensor.matmul(
                        pts[st][:],
                        r(lw_sb[:, dc, ec * P : (ec + 1) * P]),
                        r(xt_sb[:, dc, st * FT : (st + 1) * FT]),
                        start=(dc == 0),
                        stop=(dc == DC - 1),
                    )
            for st in range(ST):
                nc.vector.tensor_add(
                    xp_sb[:, ec, st * FT : (st + 1) * FT],
                    pts[st][:],
                    bias_sb[:, ec : ec + 1].to_broadcast((P, FT)),
                )
        xin.release()

        # ---- stage 2a: Q^T -> DRAM spill (W_q streamed per-column-chunk) ----
        # W_v is prefetched into the right stack now so stage 2c starts clean.
        wvp = tc.alloc_tile_pool(name="wvp", bufs=1, side="right")
        wv_sb = wvp.tile([P, DC, H], f32r, tag="wv", name="wv")
        for ec in range(DC):
            nc.sync.dma_start(
                wv_sb[:, ec, 0:FT], wv_d.ap()[ec * P : (ec + 1) * P, 0:FT]
            )
            nc.sync.dma_start(
                wv_sb[:, ec, FT:H], wv_d.ap()[ec * P : (ec + 1) * P, FT:H]
            )

        def proj_T(w_dram, out_cb):
            """out[h, s] = W.T @ xpT, h-chunk at a time; out_cb(hc, st, psum)."""
            for hc in range(HC):
                wch = wchp.tile([P, DC, P], f32r, tag="wch", name="wch")
                for ec in range(DC):
                    nc.sync.dma_start(
                        wch[:, ec, :],
                        w_dram.ap()[ec * P : (ec + 1) * P, hc * P : (hc + 1) * P],
                    )
                pts = [psmm.tile([P, FT], f32, tag="mm", name="mm") for _ in range(ST)]
                for ec in range(DC):
                    for st in range(ST):
                        nc.tensor.matmul(
                            pts[st][:],
                            r(wch[:, ec, :]),
                            r(xp_sb[:, ec, st * FT : (st + 1) * FT]),
                            start=(ec == 0),
                            stop=(ec == DC - 1),
                        )
                for st in range(ST):
                    out_cb(hc, st, pts[st])

        def q_out(hc, st, pt):
            qstg = stg.tile([P, FT], f32r, tag="stg", name="stg")
            nc.vector.tensor_copy(qstg[:], pt[:])
            nc.sync.dma_start(
                qt_spill.ap()[hc * P : (hc + 1) * P, st * FT : (st + 1) * FT],
                qstg[:],
            )

        proj_T(wq_d, q_out)

        # ---- stage 2b: K^T resident + K cache out ----
        ktp = tc.alloc_tile_pool(name="ktp", bufs=1, side="right")
        kt_sb = ktp.tile([P, HC, S], f32r, tag="kt", name="kt")

        def k_out(hc, st, pt):
            nc.vector.tensor_copy(kt_sb[:, hc, st * FT : (st + 1) * FT], pt[:])
            nc.sync.dma_start(
                KT_d.ap()[hc * P : (hc + 1) * P, st * FT : (st + 1) * FT],
                kt_sb[:, hc, st * FT : (st + 1) * FT],
            )

        proj_T(wk_d, k_out)

        # ---- stage 2c: V natural + cache out + bf16 copy ----
        wchp.release()
        vbfp = tc.alloc_tile_pool(name="vbfp", bufs=1, side="right")
        v_bf = vbfp.tile([P, QC, H], bf16, tag="vbf", name="vbf")
        for sc in range(QC):
            pts = [psmm.tile([P, FT], f32, tag="mm", name="mm") for _ in range(2)]
            for ec in range(DC):
                for ht in range(2):
                    nc.tensor.matmul(
                        pts[ht][:],
                        r(xp_sb[:, ec, sc * P : (sc + 1) * P]),
                        r(wv_sb[:, ec, ht * FT : (ht + 1) * FT]),
                        start=(ec == 0),
                        stop=(ec == DC - 1),
                    )
            for ht in range(2):
                vstg = stg.tile([P, FT], f32r, tag="stg", name="stg")
                nc.vector.tensor_copy(vstg[:], pts[ht][:])
                nc.sync.dma_start(
                    V_d.ap()[sc * P : (sc + 1) * P, ht * FT : (ht + 1) * FT],
                    vstg[:],
                )
                nc.scalar.copy(v_bf[:, sc, ht * FT : (ht + 1) * FT], pts[ht][:])

        # ---- attention, one 128-query chunk at a time, DESCENDING ----
        stg.release()
        xptp.release()
        psmm.release()
        with (
            tc.tile_pool(name="small", bufs=1) as small,
            tc.tile_pool(name="qtp", bufs=2) as qtp,
            tc.tile_pool(name="pbfp", bufs=2) as pbfp,
            tc.tile_pool(name="ptp", bufs=2) as ptp,
            tc.tile_pool(name="fp", bufs=2) as fp,
            tc.tile_pool(name="smp", bufs=3) as smp,
            tc.tile_pool(name="psS", bufs=4, space="PSUM") as psS,
            tc.tile_pool(name="psF", bufs=2, space="PSUM") as psF,
            tc.tile_pool(name="psT", bufs=2, space="PSUM") as psT,
        ):
            mask_sb = small.tile([P, 4, FT], f32, tag="mask", name="mask")
            for v in range(4):
                nc.sync.dma_start(mask_sb[:, v, :], mask_d.ap()[v])
            ident = small.tile([P, P], bf16, tag="ident", name="ident")
            make_identity(nc, ident[:])

            def qk_block(qi):
                n_kt = qi // 4 + 1
                qt = qtp.tile([P, HC, P], f32r, tag="qt", name="qt")
                for hc in range(HC):
                    nc.sync.dma_start(
                        qt[:, hc, :],
                        qt_spill.ap()[hc * P : (hc + 1) * P, qi * P : (qi + 1) * P],
                    )
                sts = [
                    psS.tile([P, FT], f32, tag="S", name="S") for _ in range(n_kt)
                ]
                for kt in range(n_kt):
                    for hc in range(HC):
                        nc.tensor.matmul(
                            sts[kt][:],
                            r(qt[:, hc, :]),
                            r(kt_sb[:, hc, kt * FT : (kt + 1) * FT]),
                            start=(hc == 0),
                            stop=(hc == HC - 1),
                        )
                return sts

            def softmax_block(qi, sts):
                n_kt = len(sts)
                v = qi % 4
                nc.vector.tensor_add(sts[-1][:], sts[-1][:], mask_sb[:, v, :])
                sm = smp.tile([P, 16], f32, tag="sm", name="sm")
                for kt in range(n_kt):
                    nc.vector.reduce_max(sm[:, kt : kt + 1], sts[kt][:], axis=AX)
                negm = sm[:, 8:9]
                nc.vector.reduce_max(negm, sm[:, :n_kt], axis=AX, negate=True)
                p_bf = pbfp.tile([P, S], bf16, tag="pbf", name="pbf")
                for kt in range(n_kt):
                    nc.scalar.activation(
                        p_bf[:, kt * FT : (kt + 1) * FT],
                        sts[kt][:],
                        EXP,
                        bias=negm,
                        accum_out=sm[:, 4 + kt : 5 + kt],
                    )
                recip = sm[:, 10:11]
                if n_kt > 1:
                    nc.vector.reduce_sum(sm[:, 9:10], sm[:, 4 : 4 + n_kt], axis=AX)
                    nc.vector.reciprocal(recip, sm[:, 9:10])
                else:
                    nc.vector.reciprocal(recip, sm[:, 4:5])
                ptb = ptp.tile([P, QC, P], bf16, tag="pt", name="pt")
                for kc in range(qi + 1):
                    tp = psT.tile([P, P], bf16, tag="tp", name="tp")
                    nc.tensor.transpose(
                        tp[:], p_bf[:, kc * P : (kc + 1) * P], ident[:]
                    )
                    nc.vector.tensor_copy(ptb[:, kc, :], tp[:])
                return ptb, recip

            def pv_block(qi, ptb, recip):
                fts = [psF.tile([P, FT], f32, tag="F", name="F") for _ in range(2)]
                for kc in range(qi + 1):
                    for ht in range(2):
                        nc.tensor.matmul(
                            fts[ht][:],
                            ptb[:, kc, :],
                            v_bf[:, kc, ht * FT : (ht + 1) * FT],
                            start=(kc == 0),
                            stop=(kc == qi),
                        )
                fsb = fp.tile([P, H], f32, tag="fsb", name="fsb")
                for ht in range(2):
                    nc.vector.tensor_mul(
                        fsb[:, ht * FT : (ht + 1) * FT],
                        fts[ht][:],
                        recip.to_broadcast((P, FT)),
                    )
                nc.sync.dma_start(F_d.ap()[qi * P : (qi + 1) * P, :], fsb[:])

            pending = None
            for qi in range(QC - 1, -1, -1):
                sts = qk_block(qi)
                ptb, recip = softmax_block(qi, sts)
                if pending is not None:
                    pv_block(*pending)
                pending = (qi, ptb, recip)
            pv_block(*pending)
        vbfp.release()
        ktp.release()
        wvp.release()

    nc.compile()
    return nc


def _get_nc():
    if "nc" not in _cache:
        _cache["nc"] = _build()
    return _cache["nc"]


def _masks():
    m = np.full((4, P, FT), NEG, dtype=np.float32)
    j = np.arange(FT)[None, :]
    p = np.arange(P)[:, None]
    for v in range(4):
        m[v][j <= p + P * v] = 0.0
    return m


_last_in_maps = None


def kernel(x_batch, lin_w, lin_b, W_q, W_k, W_v):
    from concourse.bass_utils import run_bass_kernel_spmd

    nc = _get_nc()
    x_batch = np.asarray(x_batch, dtype=np.float32)
    lwT = np.ascontiguousarray(np.asarray(lin_w, dtype=np.float32).T)
    lb = np.ascontiguousarray(
        np.asarray(lin_b, dtype=np.float32).reshape(DC, P).T
    )  # [P, DC]: lb[p, ec] = lin_b[ec*128+p]
    wq = np.ascontiguousarray(np.asarray(W_q, dtype=np.float32))
    wk = np.ascontiguousarray(np.asarray(W_k, dtype=np.float32))
    wv = np.ascontiguousarray(np.asarray(W_v, dtype=np.float32))
    masks = _masks()

    in_maps = []
    for c in range(NB):
        in_maps.append(
            {
                "xT": np.ascontiguousarray(x_batch[c].T),
                "lwT": lwT,
                "lin_b": lb,
                "W_q": wq,
                "W_k": wk,
                "W_v": wv,
                "masks": masks,
            }
        )
    global _last_in_maps
    _last_in_maps = in_maps
    res = run_bass_kernel_spmd(nc, in_maps, core_ids=list(range(NB)))
    F = np.stack([res.results[c]["F_out"] for c in range(NB)])
    K = np.stack([np.ascontiguousarray(res.results[c]["KT_out"].T) for c in range(NB)])
    V = np.stack([res.results[c]["V_out"] for c in range(NB)])
    cache = np.stack([K, V])
    return (F, cache)


# revision 8
# speedup vs baseline: 1.0134x; 1.0134x over previous
"""TRN2 Bass kernel for nn_Attention_1709396984084.

Reference computation (per batch element b, 8 of them -> one NeuronCore each):
    x  = x_b @ lin_w.T + lin_b          # [S, D]
    Q  = x @ W_q ; K = x @ W_k ; V = x @ W_v
    I  = Q @ K.T  (causal masked, NO 1/sqrt(d) scaling)
    F  = softmax(I) @ V
    returns (F, stack([K, V]))

Layout strategy (hardcoded for S=2048, D=H=1024, B=8, data-parallel over batch):
  - host passes xT = x_b.T [D, S] and lwT = lin_w.T so stage 1 needs no
    on-device transposes; stage 1 produces xpT [e, s] (e on partitions),
    which feeds the Q/K projections as moving operand (-> Q^T, K^T layouts)
    and the V projection as stationary operand (-> V natural layout).
  - Q^T spills to DRAM and streams back per 128-query chunk (SBUF capacity);
    K^T [h, s] f32 and V bf16 [s, h] stay resident.
  - scores = Q^T.T @ K^T land [q, k] in PSUM; row softmax = free-dim
    reduce_max + ScalarE exp with per-partition -max bias and fused row-sum
    (accum_out); P~ cast to bf16, transposed 128x128 on TensorE (identity),
    then P~^T @ V accumulates F; 1/rowsum applied on the way out.
  - matmul dtypes: float32r (fp32 storage, ~1.5e-4 matmul rel err, bf16-rate
    at N=512) for the logit-sensitive chain; bf16 for P@V.
  - attention runs q-chunks in DESCENDING order (big chunks first) with a
    one-chunk software pipeline so P@V of chunk i fills the PE while the
    softmax of chunk i-1 runs on DVE/ACT.
Measured: relF ~2.8e-3, relK/V ~2e-4 vs the fp32 reference.
"""

import sys

sys.path.insert(0, "/opt/trn_rl_repo")

import numpy as np

P = 128
S = 2048  # sequence length
D = 1024  # input size
H = 1024  # hidden size
FT = 512  # free-dim tile (one PSUM bank of fp32)
NB = 8  # batch == number of cores
DC = D // P  # 8 contraction chunks
HC = H // P
ST = S // FT  # 4 s-tiles
QC = S // P  # 16 query chunks
NEG = -1.0e30

_cache = {}


def _build():
    import concourse.mybir as mybir
    import concourse.tile as tile
    from concourse import bacc
    from concourse.masks import make_identity

    f32 = mybir.dt.float32
    f32r = mybir.dt.float32r
    bf16 = mybir.dt.bfloat16
    EXP = mybir.ActivationFunctionType.Exp
    AX = mybir.AxisListType.X

    def r(ap):
        return ap

    nc = bacc.Bacc(None, target_bir_lowering=False)

    xT_d = nc.declare_dram_parameter("xT", [D, S], f32r, isOutput=False)
    lwT_d = nc.declare_dram_parameter("lwT", [D, D], f32r, isOutput=False)
    lb_d = nc.declare_dram_parameter("lin_b", [P, DC], f32, isOutput=False)
    wq_d = nc.declare_dram_parameter("W_q", [D, H], f32r, isOutput=False)
    wk_d = nc.declare_dram_parameter("W_k", [D, H], f32r, isOutput=False)
    wv_d = nc.declare_dram_parameter("W_v", [D, H], f32r, isOutput=False)
    mask_d = nc.declare_dram_parameter("masks", [4, P, FT], f32, isOutput=False)
    F_d = nc.declare_dram_parameter("F_out", [S, H], f32, isOutput=True)
    KT_d = nc.declare_dram_parameter("KT_out", [H, S], f32r, isOutput=True)
    V_d = nc.declare_dram_parameter("V_out", [S, H], f32r, isOutput=True)
    qt_spill = nc.dram_tensor("QT_spill", [H, S], f32r)

    with tile.TileContext(nc) as tc:
        qtp = tc.alloc_tile_pool(name="qtp", bufs=2)
        xptp = tc.alloc_tile_pool(name="xptp", bufs=1)
        stg = tc.alloc_tile_pool(name="stg", bufs=2)
        wchp = tc.alloc_tile_pool(name="wchp", bufs=8)
        psmm = tc.alloc_tile_pool(name="psmm", bufs=8, space="PSUM")
        xp_sb = xptp.tile([P, DC, S], f32r, tag="xpt", name="xpt")

        # ---- stage 1: xpT[e, s] = lin_w @ x.T + b ----
        xin = tc.alloc_tile_pool(name="xin", bufs=1)
        xt_sb = xin.tile([P, DC, S], f32r, tag="xt", name="xt")
        lw_sb = xin.tile([P, DC, D], f32r, tag="lwt", name="lwt")
        bias_sb = xin.tile([P, DC], f32, tag="bias", name="bias")
        nc.sync.dma_start(bias_sb[:], lb_d.ap())
        for dc in range(DC):
            for st in range(ST):  # split across queues for early arrival
                nc.sync.dma_start(
                    xt_sb[:, dc, st * FT : (st + 1) * FT],
                    xT_d.ap()[dc * P : (dc + 1) * P, st * FT : (st + 1) * FT],
                )
            nc.sync.dma_start(
                lw_sb[:, dc, 0:FT], lwT_d.ap()[dc * P : (dc + 1) * P, 0:FT]
            )
            nc.sync.dma_start(
                lw_sb[:, dc, FT:D], lwT_d.ap()[dc * P : (dc + 1) * P, FT:D]
            )
        for ec in range(DC):
            pts = [psmm.tile([P, FT], f32, tag="mm", name="mm") for _ in range(ST)]
            for dc in range(DC):
                for st in range(ST):
                    nc.tensor.matmul(
                        pts[st][:],
                        r(lw_sb[:, dc, ec * P : (ec + 1) * P]),
                        r(xt_sb[:, dc, st * FT : (st + 1) * FT]),
                        start=(dc == 0),
                        stop=(dc == DC - 1),
                    )
            for st in range(ST):
                nc.vector.tensor_add(
                    xp_sb[:, ec, st * FT : (st + 1) * FT],
                    pts[st][:],
                    bias_sb[:, ec : ec + 1].to_broadcast((P, FT)),
                )
        xin.release()

        # ---- stage 2a: Q^T -> DRAM spill (W_q streamed per-column-chunk) ----
        # W_v is prefetched into the right stack now so stage 2c starts clean.
        wvp = tc.alloc_tile_pool(name="wvp", bufs=1, side="right")
        wv_sb = wvp.tile([P, DC, H], f32r, tag="wv", name="wv")
        for ec in range(DC):
            nc.sync.dma_start(
                wv_sb[:, ec, 0:FT], wv_d.ap()[ec * P : (ec + 1) * P, 0:FT]
            )
            nc.sync.dma_start(
                wv_sb[:, ec, FT:H], wv_d.ap()[ec * P : (ec + 1) * P, FT:H]
            )

        def proj_T(w_dram, out_cb):
            """out[h, s] = W.T @ xpT, h-chunk at a time; out_cb(hc, st, psum)."""
            for hc in range(HC):
                wch = wchp.tile([P, DC, P], f32r, tag="wch", name="wch")
                for ec in range(DC):
                    nc.sync.dma_start(
                        wch[:, ec, :],
                        w_dram.ap()[ec * P : (ec + 1) * P, hc * P : (hc + 1) * P],
                    )
                pts = [psmm.tile([P, FT], f32, tag="mm", name="mm") for _ in range(ST)]
                for ec in range(DC):
                    for st in range(ST):
                        nc.tensor.matmul(
                            pts[st][:],
                            r(wch[:, ec, :]),
                            r(xp_sb[:, ec, st * FT : (st + 1) * FT]),
                            start=(ec == 0),
                            stop=(ec == DC - 1),
                        )
                for st in range(ST):
                    out_cb(hc, st, pts[st])

        def q_out(hc, st, pt):
            qstg = stg.tile([P, FT], f32r, tag="stg", name="stg")
            nc.vector.tensor_copy(qstg[:], pt[:])
            nc.sync.dma_start(
                qt_spill.ap()[hc * P : (hc + 1) * P, st * FT : (st + 1) * FT],
                qstg[:],
            )

        proj_T(wq_d, q_out)

        # prefetch the first two attention q-chunks' Q^T columns now, so the
        # attention phase doesn't queue behind the K/V output DMA traffic
        qt_tiles = {}

        def load_qt(qi):
            qt = qtp.tile([P, HC, P], f32r, tag="qt", name="qt")
            for hc in range(HC):
                nc.sync.dma_start(
                    qt[:, hc, :],
                    qt_spill.ap()[hc * P : (hc + 1) * P, qi * P : (qi + 1) * P],
                )
            qt_tiles[qi] = qt

        load_qt(QC - 1)
        load_qt(QC - 2)

        # ---- stage 2b: K^T resident + K cache out ----
        ktp = tc.alloc_tile_pool(name="ktp", bufs=1, side="right")
        kt_sb = ktp.tile([P, HC, S], f32r, tag="kt", name="kt")

        def k_out(hc, st, pt):
            nc.vector.tensor_copy(kt_sb[:, hc, st * FT : (st + 1) * FT], pt[:])
            nc.sync.dma_start(
                KT_d.ap()[hc * P : (hc + 1) * P, st * FT : (st + 1) * FT],
                kt_sb[:, hc, st * FT : (st + 1) * FT],
            )

        proj_T(wk_d, k_out)

        # ---- stage 2c: V natural + cache out + bf16 copy ----
        wchp.release()
        vbfp = tc.alloc_tile_pool(name="vbfp", bufs=1, side="right")
        v_bf = vbfp.tile([P, QC, H], bf16, tag="vbf", name="vbf")
        for sc in range(QC):
            pts = [psmm.tile([P, FT], f32, tag="mm", name="mm") for _ in range(2)]
            for ec in range(DC):
                for ht in range(2):
                    nc.tensor.matmul(
                        pts[ht][:],
                        r(xp_sb[:, ec, sc * P : (sc + 1) * P]),
                        r(wv_sb[:, ec, ht * FT : (ht + 1) * FT]),
                        start=(ec == 0),
                        stop=(ec == DC - 1),
                    )
            for ht in range(2):
                vstg = stg.tile([P, FT], f32r, tag="stg", name="stg")
                nc.vector.tensor_copy(vstg[:], pts[ht][:])
                nc.sync.dma_start(
                    V_d.ap()[sc * P : (sc + 1) * P, ht * FT : (ht + 1) * FT],
                    vstg[:],
                )
                nc.scalar.copy(v_bf[:, sc, ht * FT : (ht + 1) * FT], pts[ht][:])

        # ---- attention, one 128-query chunk at a time, DESCENDING ----
        stg.release()
        xptp.release()
        psmm.release()
        with (
            tc.tile_pool(name="small", bufs=1) as small,
            tc.tile_pool(name="pbfp", bufs=2) as pbfp,
            tc.tile_pool(name="ptp", bufs=2) as ptp,
            tc.tile_pool(name="fp", bufs=2) as fp,
            tc.tile_pool(name="smp", bufs=3) as smp,
            tc.tile_pool(name="psS", bufs=6, space="PSUM") as psS,
            tc.tile_pool(name="psF", bufs=2, space="PSUM") as psF,
        ):
            mask_sb = small.tile([P, 4, FT], f32, tag="mask", name="mask")
            for v in range(4):
                nc.sync.dma_start(mask_sb[:, v, :], mask_d.ap()[v])
            ident = small.tile([P, P], bf16, tag="ident", name="ident")
            make_identity(nc, ident[:])

            def qk_block(qi):
                n_kt = qi // 4 + 1
                if qi - 2 >= 0:
                    load_qt(qi - 2)
                qt = qt_tiles.pop(qi)
                sts = [
                    psS.tile([P, FT], f32, tag="S", name="S") for _ in range(n_kt)
                ]
                for kt in range(n_kt):
                    for hc in range(HC):
                        nc.tensor.matmul(
                            sts[kt][:],
                            r(qt[:, hc, :]),
                            r(kt_sb[:, hc, kt * FT : (kt + 1) * FT]),
                            start=(hc == 0),
                            stop=(hc == HC - 1),
                        )
                return sts

            def softmax_block(qi, sts):
                n_kt = len(sts)
                v = qi % 4
                nc.vector.tensor_add(sts[-1][:], sts[-1][:], mask_sb[:, v, :])
                sm = smp.tile([P, 16], f32, tag="sm", name="sm")
                for kt in range(n_kt):
                    nc.vector.reduce_max(sm[:, kt : kt + 1], sts[kt][:], axis=AX)
                negm = sm[:, 8:9]
                nc.vector.reduce_max(negm, sm[:, :n_kt], axis=AX, negate=True)
                p_bf = pbfp.tile([P, S], bf16, tag="pbf", name="pbf")
                for kt in range(n_kt):
                    nc.scalar.activation(
                        p_bf[:, kt * FT : (kt + 1) * FT],
                        sts[kt][:],
                        EXP,
                        bias=negm,
                        accum_out=sm[:, 4 + kt : 5 + kt],
                    )
                recip = sm[:, 10:11]
                if n_kt > 1:
                    nc.vector.reduce_sum(sm[:, 9:10], sm[:, 4 : 4 + n_kt], axis=AX)
                    nc.vector.reciprocal(recip, sm[:, 9:10])
                else:
                    nc.vector.reciprocal(recip, sm[:, 4:5])
                ptb = ptp.tile([P, QC, P], bf16, tag="pt", name="pt")
                for kc in range(qi + 1):
                    tp = psS.tile([P, P], bf16, tag="S", name="S_tp")
                    nc.tensor.transpose(
                        tp[:], p_bf[:, kc * P : (kc + 1) * P], ident[:]
                    )
                    nc.vector.tensor_copy(ptb[:, kc, :], tp[:])
                return ptb, recip

            def pv_block(qi, ptb, recip):
                fts = [psF.tile([P, FT], f32, tag="F", name="F") for _ in range(2)]
                for kc in range(qi + 1):
                    for ht in range(2):
                        nc.tensor.matmul(
                            fts[ht][:],
                            ptb[:, kc, :],
                            v_bf[:, kc, ht * FT : (ht + 1) * FT],
                            start=(kc == 0),
                            stop=(kc == qi),
                        )
                fsb = fp.tile([P, H], f32, tag="fsb", name="fsb")
                for ht in range(2):
                    nc.vector.tensor_mul(
                        fsb[:, ht * FT : (ht + 1) * FT],
                        fts[ht][:],
                        recip.to_broadcast((P, FT)),
                    )
                nc.sync.dma_start(F_d.ap()[qi * P : (qi + 1) * P, :], fsb[:])

            pending = None
            for qi in range(QC - 1, -1, -1):
                sts = qk_block(qi)
                ptb, recip = softmax_block(qi, sts)
                if pending is not None:
                    pv_block(*pending)
                pending = (qi, ptb, recip)
            pv_block(*pending)
        qtp.release()
        vbfp.release()
        ktp.release()
        wvp.release()

    nc.compile()
    return nc


def _get_nc():
    if "nc" not in _cache:
        _cache["nc"] = _build()
    return _cache["nc"]


def _masks():
    m = np.full((4, P, FT), NEG, dtype=np.float32)
    j = np.arange(FT)[None, :]
    p = np.arange(P)[:, None]
    for v in range(4):
        m[v][j <= p + P * v] = 0.0
    return m


_last_in_maps = None


def kernel(x_batch, lin_w, lin_b, W_q, W_k, W_v):
    from concourse.bass_utils import run_bass_kernel_spmd

    nc = _get_nc()
    x_batch = np.asarray(x_batch, dtype=np.float32)
    lwT = np.ascontiguousarray(np.asarray(lin_w, dtype=np.float32).T)
    lb = np.ascontiguousarray(
        np.asarray(lin_b, dtype=np.float32).reshape(DC, P).T
    )  # [P, DC]: lb[p, ec] = lin_b[ec*128+p]
    wq = np.ascontiguousarray(np.asarray(W_q, dtype=np.float32))
    wk = np.ascontiguousarray(np.asarray(W_k, dtype=np.float32))
    wv = np.ascontiguousarray(np.asarray(W_v, dtype=np.float32))
    masks = _masks()

    in_maps = []
    for c in range(NB):
        in_maps.append(
            {
                "xT": np.ascontiguousarray(x_batch[c].T),
                "lwT": lwT,
                "lin_b": lb,
                "W_q": wq,
                "W_k": wk,
                "W_v": wv,
                "masks": masks,
            }
        )
    global _last_in_maps
    _last_in_maps = in_maps
    res = run_bass_kernel_spmd(nc, in_maps, core_ids=list(range(NB)))
    F = np.stack([res.results[c]["F_out"] for c in range(NB)])
    K = np.stack([np.ascontiguousarray(res.results[c]["KT_out"].T) for c in range(NB)])
    V = np.stack([res.results[c]["V_out"] for c in range(NB)])
    cache = np.stack([K, V])
    return (F, cache)


# revision 9
# speedup vs baseline: 1.0236x; 1.0100x over previous
"""TRN2 Bass kernel for nn_Attention_1709396984084.

Reference computation (per batch element b, 8 of them -> one NeuronCore each):
    x  = x_b @ lin_w.T + lin_b          # [S, D]
    Q  = x @ W_q ; K = x @ W_k ; V = x @ W_v
    I  = Q @ K.T  (causal masked, NO 1/sqrt(d) scaling)
    F  = softmax(I) @ V
    returns (F, stack([K, V]))

Layout strategy (hardcoded for S=2048, D=H=1024, B=8, data-parallel over batch):
  - host passes xT = x_b.T [D, S] and lwT = lin_w.T so stage 1 needs no
    on-device transposes; stage 1 produces xpT [e, s] (e on partitions),
    which feeds the Q/K projections as moving operand (-> Q^T, K^T layouts)
    and the V projection as stationary operand (-> V natural layout).
  - Q^T spills to DRAM and streams back per 128-query chunk (SBUF capacity);
    K^T [h, s] f32 and V bf16 [s, h] stay resident.
  - scores = Q^T.T @ K^T land [q, k] in PSUM; row softmax = free-dim
    reduce_max + ScalarE exp with per-partition -max bias and fused row-sum
    (accum_out); P~ cast to bf16, transposed 128x128 on TensorE (identity),
    then P~^T @ V accumulates F; 1/rowsum applied on the way out.
  - matmul dtypes: float32r (fp32 storage, ~1.5e-4 matmul rel err, bf16-rate
    at N=512) for the logit-sensitive chain; bf16 for P@V.
  - attention runs q-chunks in DESCENDING order (big chunks first) with a
    one-chunk software pipeline so P@V of chunk i fills the PE while the
    softmax of chunk i-1 runs on DVE/ACT.
Measured: relF ~2.8e-3, relK/V ~2e-4 vs the fp32 reference.
"""

import sys

sys.path.insert(0, "/opt/trn_rl_repo")

import numpy as np

P = 128
S = 2048  # sequence length
D = 1024  # input size
H = 1024  # hidden size
FT = 512  # free-dim tile (one PSUM bank of fp32)
NB = 8  # batch == number of cores
DC = D // P  # 8 contraction chunks
HC = H // P
ST = S // FT  # 4 s-tiles
QC = S // P  # 16 query chunks
NEG = -1.0e30

_cache = {}


def _build():
    import concourse.mybir as mybir
    import concourse.tile as tile
    from concourse import bacc
    from concourse.masks import make_identity

    f32 = mybir.dt.float32
    f32r = mybir.dt.float32r
    bf16 = mybir.dt.bfloat16
    EXP = mybir.ActivationFunctionType.Exp
    AX = mybir.AxisListType.X

    def r(ap):
        return ap

    nc = bacc.Bacc(None, target_bir_lowering=False)

    xT_d = nc.declare_dram_parameter("xT", [D, S], f32r, isOutput=False)
    lwT_d = nc.declare_dram_parameter("lwT", [D, D], f32r, isOutput=False)
    lb_d = nc.declare_dram_parameter("lin_b", [P, DC], f32, isOutput=False)
    wq_d = nc.declare_dram_parameter("W_q", [D, H], f32r, isOutput=False)
    wk_d = nc.declare_dram_parameter("W_k", [D, H], f32r, isOutput=False)
    wv_d = nc.declare_dram_parameter("W_v", [D, H], f32r, isOutput=False)
    mask_d = nc.declare_dram_parameter("masks", [4, P, FT], f32, isOutput=False)
    F_d = nc.declare_dram_parameter("F_out", [S, H], f32, isOutput=True)
    KT_d = nc.declare_dram_parameter("KT_out", [H, S], f32r, isOutput=True)
    V_d = nc.declare_dram_parameter("V_out", [S, H], f32r, isOutput=True)
    qt_spill = nc.dram_tensor("QT_spill", [H, S], f32r)

    with tile.TileContext(nc) as tc:
        qtp = tc.alloc_tile_pool(name="qtp", bufs=2)
        xptp = tc.alloc_tile_pool(name="xptp", bufs=1)
        stg = tc.alloc_tile_pool(name="stg", bufs=3)
        wchp = tc.alloc_tile_pool(name="wchp", bufs=7)
        psmm = tc.alloc_tile_pool(name="psmm", bufs=8, space="PSUM")
        xp_sb = xptp.tile([P, DC, S], f32r, tag="xpt", name="xpt")

        # ---- stage 1: xpT[e, s] = lin_w @ x.T + b ----
        xin = tc.alloc_tile_pool(name="xin", bufs=1)
        xt_sb = xin.tile([P, DC, S], f32r, tag="xt", name="xt")
        lw_sb = xin.tile([P, DC, D], f32r, tag="lwt", name="lwt")
        bias_sb = xptp.tile([P, DC], f32, tag="bias", name="bias")
        nc.sync.dma_start(bias_sb[:], lb_d.ap())
        for dc in range(DC):
            nc.sync.dma_start(
                lw_sb[:, dc, 0:FT], lwT_d.ap()[dc * P : (dc + 1) * P, 0:FT]
            )
            nc.sync.dma_start(
                lw_sb[:, dc, FT:D], lwT_d.ap()[dc * P : (dc + 1) * P, FT:D]
            )
            for st in range(ST):  # split across queues for early arrival
                nc.sync.dma_start(
                    xt_sb[:, dc, st * FT : (st + 1) * FT],
                    xT_d.ap()[dc * P : (dc + 1) * P, st * FT : (st + 1) * FT],
                )
        for ec in range(DC):
            pts = [psmm.tile([P, FT], f32, tag="mm", name="mm") for _ in range(ST)]
            for dc in range(DC):
                for st in range(ST):
                    nc.tensor.matmul(
                        pts[st][:],
                        r(lw_sb[:, dc, ec * P : (ec + 1) * P]),
                        r(xt_sb[:, dc, st * FT : (st + 1) * FT]),
                        start=(dc == 0),
                        stop=(dc == DC - 1),
                    )
            for st in range(ST):
                nc.vector.tensor_add(
                    xp_sb[:, ec, st * FT : (st + 1) * FT],
                    pts[st][:],
                    bias_sb[:, ec : ec + 1].to_broadcast((P, FT)),
                )
        xin.release()

        # ---- stage 2a: Q^T -> DRAM spill (W_q streamed per-column-chunk) ----
        # W_v is prefetched into the right stack now so stage 2c starts clean.
        wvp = tc.alloc_tile_pool(name="wvp", bufs=1, side="right")
        wv_sb = wvp.tile([P, DC, H], f32r, tag="wv", name="wv")
        for ec in range(DC):
            nc.sync.dma_start(
                wv_sb[:, ec, 0:FT], wv_d.ap()[ec * P : (ec + 1) * P, 0:FT]
            )
            nc.sync.dma_start(
                wv_sb[:, ec, FT:H], wv_d.ap()[ec * P : (ec + 1) * P, FT:H]
            )

        def proj_T(w_dram, out_cb):
            """out[h, s] = W.T @ xpT, h-chunk at a time; out_cb(hc, st, psum)."""
            for hc in range(HC):
                wch = wchp.tile([P, DC, P], f32r, tag="wch", name="wch")
                for ec in range(DC):
                    nc.sync.dma_start(
                        wch[:, ec, :],
                        w_dram.ap()[ec * P : (ec + 1) * P, hc * P : (hc + 1) * P],
                    )
                pts = [psmm.tile([P, FT], f32, tag="mm", name="mm") for _ in range(ST)]
                for ec in range(DC):
                    for st in range(ST):
                        nc.tensor.matmul(
                            pts[st][:],
                            r(wch[:, ec, :]),
                            r(xp_sb[:, ec, st * FT : (st + 1) * FT]),
                            start=(ec == 0),
                            stop=(ec == DC - 1),
                        )
                for st in range(ST):
                    out_cb(hc, st, pts[st])

        def q_out(hc, st, pt):
            qstg = stg.tile([P, FT], f32r, tag="stg", name="stg")
            nc.vector.tensor_copy(qstg[:], pt[:])
            nc.sync.dma_start(
                qt_spill.ap()[hc * P : (hc + 1) * P, st * FT : (st + 1) * FT],
                qstg[:],
            )

        proj_T(wq_d, q_out)

        # prefetch the first two attention q-chunks' Q^T columns now, so the
        # attention phase doesn't queue behind the K/V output DMA traffic
        qt_tiles = {}

        def load_qt(qi):
            qt = qtp.tile([P, HC, P], f32r, tag="qt", name="qt")
            for hc in range(HC):
                nc.sync.dma_start(
                    qt[:, hc, :],
                    qt_spill.ap()[hc * P : (hc + 1) * P, qi * P : (qi + 1) * P],
                )
            qt_tiles[qi] = qt

        load_qt(QC - 1)
        load_qt(QC - 2)

        # ---- stage 2b: K^T resident + K cache out ----
        ktp = tc.alloc_tile_pool(name="ktp", bufs=1, side="right")
        kt_sb = ktp.tile([P, HC, S], f32r, tag="kt", name="kt")

        def k_out(hc, st, pt):
            nc.vector.tensor_copy(kt_sb[:, hc, st * FT : (st + 1) * FT], pt[:])
            nc.sync.dma_start(
                KT_d.ap()[hc * P : (hc + 1) * P, st * FT : (st + 1) * FT],
                kt_sb[:, hc, st * FT : (st + 1) * FT],
            )

        proj_T(wk_d, k_out)

        # ---- stage 2c: V natural + cache out + bf16 copy ----
        wchp.release()
        vbfp = tc.alloc_tile_pool(name="vbfp", bufs=1, side="right")
        v_bf = vbfp.tile([P, QC, H], bf16, tag="vbf", name="vbf")
        for sc in range(QC):
            pts = [psmm.tile([P, FT], f32, tag="mm", name="mm") for _ in range(2)]
            for ec in range(DC):
                for ht in range(2):
                    nc.tensor.matmul(
                        pts[ht][:],
                        r(xp_sb[:, ec, sc * P : (sc + 1) * P]),
                        r(wv_sb[:, ec, ht * FT : (ht + 1) * FT]),
                        start=(ec == 0),
                        stop=(ec == DC - 1),
                    )
            for ht in range(2):
                vstg = stg.tile([P, FT], f32r, tag="stg", name="stg")
                nc.vector.tensor_copy(vstg[:], pts[ht][:])
                nc.sync.dma_start(
                    V_d.ap()[sc * P : (sc + 1) * P, ht * FT : (ht + 1) * FT],
                    vstg[:],
                )
                nc.scalar.copy(v_bf[:, sc, ht * FT : (ht + 1) * FT], pts[ht][:])

        # ---- attention, one 128-query chunk at a time, DESCENDING ----
        stg.release()
        xptp.release()
        psmm.release()
        with (
            tc.tile_pool(name="small", bufs=1) as small,
            tc.tile_pool(name="pbfp", bufs=2) as pbfp,
            tc.tile_pool(name="ptp", bufs=2) as ptp,
            tc.tile_pool(name="fp", bufs=2) as fp,
            tc.tile_pool(name="smp", bufs=3) as smp,
            tc.tile_pool(name="psS", bufs=6, space="PSUM") as psS,
            tc.tile_pool(name="psF", bufs=2, space="PSUM") as psF,
        ):
            mask_sb = small.tile([P, 4, FT], f32, tag="mask", name="mask")
            for v in range(4):
                nc.sync.dma_start(mask_sb[:, v, :], mask_d.ap()[v])
            ident = small.tile([P, P], bf16, tag="ident", name="ident")
            make_identity(nc, ident[:])

            def qk_block(qi):
                n_kt = qi // 4 + 1
                if qi - 2 >= 0:
                    load_qt(qi - 2)
                qt = qt_tiles.pop(qi)
                sts = [
                    psS.tile([P, FT], f32, tag="S", name="S") for _ in range(n_kt)
                ]
                for kt in range(n_kt):
                    for hc in range(HC):
                        nc.tensor.matmul(
                            sts[kt][:],
                            r(qt[:, hc, :]),
                            r(kt_sb[:, hc, kt * FT : (kt + 1) * FT]),
                            start=(hc == 0),
                            stop=(hc == HC - 1),
                        )
                return sts

            def softmax_block(qi, sts):
                n_kt = len(sts)
                v = qi % 4
                nc.vector.tensor_add(sts[-1][:], sts[-1][:], mask_sb[:, v, :])
                sm = smp.tile([P, 16], f32, tag="sm", name="sm")
                for kt in range(n_kt):
                    nc.vector.reduce_max(sm[:, kt : kt + 1], sts[kt][:], axis=AX)
                negm = sm[:, 8:9]
                nc.vector.reduce_max(negm, sm[:, :n_kt], axis=AX, negate=True)
                p_bf = pbfp.tile([P, S], bf16, tag="pbf", name="pbf")
                for kt in range(n_kt):
                    nc.scalar.activation(
                        p_bf[:, kt * FT : (kt + 1) * FT],
                        sts[kt][:],
                        EXP,
                        bias=negm,
                        accum_out=sm[:, 4 + kt : 5 + kt],
                    )
                recip = sm[:, 10:11]
                if n_kt > 1:
                    nc.vector.reduce_sum(sm[:, 9:10], sm[:, 4 : 4 + n_kt], axis=AX)
                    nc.vector.reciprocal(recip, sm[:, 9:10])
                else:
                    nc.vector.reciprocal(recip, sm[:, 4:5])
                ptb = ptp.tile([P, QC, P], bf16, tag="pt", name="pt")
                for kc in range(qi + 1):
                    tp = psS.tile([P, P], bf16, tag="S", name="S_tp")
                    nc.tensor.transpose(
                        tp[:], p_bf[:, kc * P : (kc + 1) * P], ident[:]
                    )
                    nc.vector.tensor_copy(ptb[:, kc, :], tp[:])
                return ptb, recip

            def pv_block(qi, ptb, recip):
                fts = [psF.tile([P, FT], f32, tag="F", name="F") for _ in range(2)]
                for kc in range(qi + 1):
                    for ht in range(2):
                        nc.tensor.matmul(
                            fts[ht][:],
                            ptb[:, kc, :],
                            v_bf[:, kc, ht * FT : (ht + 1) * FT],
                            start=(kc == 0),
                            stop=(kc == qi),
                        )
                fsb = fp.tile([P, H], f32, tag="fsb", name="fsb")
                for ht in range(2):
                    nc.vector.tensor_mul(
                        fsb[:, ht * FT : (ht + 1) * FT],
                        fts[ht][:],
                        recip.to_broadcast((P, FT)),
                    )
                nc.sync.dma_start(F_d.ap()[qi * P : (qi + 1) * P, :], fsb[:])

            pending = None
            for qi in range(QC - 1, 5, -1):
                sts = qk_block(qi)
                ptb, recip = softmax_block(qi, sts)
                if pending is not None:
                    pv_block(*pending)
                pending = (qi, ptb, recip)
            for a in (5, 3, 1):
                b = a - 1
                sts_a = qk_block(a)
                sts_b = qk_block(b)
                ptb_a, recip_a = softmax_block(a, sts_a)
                if pending is not None:
                    pv_block(*pending)
                ptb_b, recip_b = softmax_block(b, sts_b)
                pv_block(a, ptb_a, recip_a)
                pending = (b, ptb_b, recip_b)
            pv_block(*pending)
        qtp.release()
        vbfp.release()
        ktp.release()
        wvp.release()

    nc.compile()
    return nc


def _get_nc():
    if "nc" not in _cache:
        _cache["nc"] = _build()
    return _cache["nc"]


def _masks():
    m = np.full((4, P, FT), NEG, dtype=np.float32)
    j = np.arange(FT)[None, :]
    p = np.arange(P)[:, None]
    for v in range(4):
        m[v][j <= p + P * v] = 0.0
    return m


_last_in_maps = None


def kernel(x_batch, lin_w, lin_b, W_q, W_k, W_v):
    from concourse.bass_utils import run_bass_kernel_spmd

    nc = _get_nc()
    x_batch = np.asarray(x_batch, dtype=np.float32)
    lwT = np.ascontiguousarray(np.asarray(lin_w, dtype=np.float32).T)
    lb = np.ascontiguousarray(
        np.asarray(lin_b, dtype=np.float32).reshape(DC, P).T
    )  # [P, DC]: lb[p, ec] = lin_b[ec*128+p]
    wq = np.ascontiguousarray(np.asarray(W_q, dtype=np.float32))
    wk = np.ascontiguousarray(np.asarray(W_k, dtype=np.float32))
    wv = np.ascontiguousarray(np.asarray(W_v, dtype=np.float32))
    masks = _masks()

    in_maps = []
    for c in range(NB):
        in_maps.append(
            {
                "xT": np.ascontiguousarray(x_batch[c].T),
                "lwT": lwT,
                "lin_b": lb,
                "W_q": wq,
                "W_k": wk,
                "W_v": wv,
                "masks": masks,
            }
        )
    global _last_in_maps
    _last_in_maps = in_maps
    res = run_bass_kernel_spmd(nc, in_maps, core_ids=list(range(NB)))
    F = np.stack([res.results[c]["F_out"] for c in range(NB)])
    K = np.stack([np.ascontiguousarray(res.results[c]["KT_out"].T) for c in range(NB)])
    V = np.stack([res.results[c]["V_out"] for c in range(NB)])
    cache = np.stack([K, V])
    return (F, cache)
